# revision 4
# baseline (speedup 1.0000x reference)
"""DUQ RBF head kernel for Trainium2 (8 NeuronCores, batch-parallel).

Computes out[b,c,h,w] = exp(gamma * mean_e (einsum('bfhw,ecf', x, W) - m/N)^2)
for features [8,512,128,128], weights [16,64,512], m [16,64], N [64].

Strategy: data-parallel over batch (1 image per core). Per core, one big
matmul [ec=1024, f=512] @ [f=512, pix=16384] in float32r (full-rate fp32,
self-loading weights; LDWEIGHTS hides behind the previous matmul's
streaming). Pixels are processed in groups of up to 1024 (two 512-col
PSUM banks per ec-chunk); the Square epilogue folds the centroid into the
per-partition ACT bias. N per matmul stays >= 256: fp32r drops to 1/4
rate below 256 moving columns.

Schedule shaping:
- Groups taper up [256, 512, 1024...] so the first matmul only waits for
  a 512KB x-group and one 256KB weight tile, and taper down
  [..., 512, 512, 256] so the drain after the last matmul is short.
- Startup DMAs ride three parallel paths (scalar HWDGE: x groups 0-1;
  sync HWDGE: even weight tiles + even x groups; gpsimd SWDGE: odd
  weight tiles + odd x groups) as whole tiles: small per-k slices lose
  to the ~1us fixed DMA latency and starve the PE.
- The 8 squared ec-chunks accumulate as a 2-engine tree (pairs 0-3 on
  DVE, pairs 4-7 on GpSimd, combine on DVE) instead of a serial 7-add
  chain, so the post-matmul epilogue latency at the kernel tail is a
  ~6-step tree, not a 15-step ladder.
"""

import numpy as np

import concourse.bacc as bacc_mod
import concourse.mybir as mybir
import concourse.tile as tile
from concourse.bass_utils import run_bass_kernel_spmd

dt = mybir.dt
Act = mybir.ActivationFunctionType

B, F, H, W = 8, 512, 128, 128
E, C = 16, 64
PIX = H * W           # 16384 pixels per image
MCH = (E * C) // 128  # 8 ec-chunks of 128 partitions
KCH = F // 128        # 4 contraction chunks
LENGTH_SCALE = 0.1
GAMMA = -1.0 / (2.0 * LENGTH_SCALE**2)   # -50.0
EXP_SCALE = GAMMA / E                    # -3.125

GROUPS = [256, 512] + [1024] * 14 + [512, 512, 256]
assert sum(GROUPS) == PIX
assert all(g >= 256 for g in GROUPS)  # fp32r is 1/4-rate below 256 cols
GW = 1024  # allocation width; narrower groups use [:, :gw] slices


def _build():
    nc = bacc_mod.Bacc(None)
    feat_d = nc.declare_dram_parameter("feat", [F, PIX], dt.float32r, isOutput=False)
    wt_d = nc.declare_dram_parameter("wt", [F, E * C], dt.float32r, isOutput=False)
    negc_d = nc.declare_dram_parameter("negc", [128, MCH], dt.float32, isOutput=False)
    out_d = nc.declare_dram_parameter("out", [C, PIX], dt.float32, isOutput=True)

    feat_k = feat_d.rearrange("(k p) x -> p k x", k=KCH)
    wt_k = wt_d.rearrange("(k p) m -> p k m", k=KCH)

    with tile.TileContext(nc) as tc:
        with (
            tc.tile_pool(name="singles", bufs=1) as singles,
            tc.tile_pool(name="xin", bufs=4) as xin,
            tc.tile_pool(name="sqp", bufs=2) as sqp,
            tc.tile_pool(name="redp", bufs=2) as redp,
            tc.tile_pool(name="accp", bufs=2) as accp,
            tc.tile_pool(name="outp", bufs=2) as outp,
            tc.tile_pool(name="ps", bufs=4, space="PSUM") as ps,
        ):
            # negc rides the gpsimd queue ahead of the odd weight tiles
            negc_sb = singles.tile([128, MCH], dt.float32, tag="negc")
            nc.gpsimd.dma_start(out=negc_sb, in_=negc_d[:, :])

            ws = []
            for m in range(MCH):
                wsm = singles.tile([128, KCH, 128], dt.float32r, tag=f"ws{m}")
                ws.append(wsm)

            # Startup wave on three parallel DMA paths; whole tiles only.
            g0w, g1w = GROUPS[0], GROUPS[1]
            xg0 = xin.tile([128, KCH, GW], dt.float32r, tag="x")
            xg1 = xin.tile([128, KCH, GW], dt.float32r, tag="x")
            nc.sync.dma_start(out=ws[0], in_=wt_k[:, :, 0:128])
            nc.scalar.dma_start(out=xg0[:, :, 0:g0w], in_=feat_k[:, :, 0:g0w])
            nc.gpsimd.dma_start(out=ws[1], in_=wt_k[:, :, 128:256])
            for m in (2, 4, 6):
                nc.sync.dma_start(out=ws[m], in_=wt_k[:, :, m * 128 : (m + 1) * 128])
            for m in (3, 5, 7):
                nc.gpsimd.dma_start(out=ws[m], in_=wt_k[:, :, m * 128 : (m + 1) * 128])
            px1 = slice(g0w, g0w + g1w)
            nc.scalar.dma_start(out=xg1[:, 0:2, 0:g1w], in_=feat_k[:, 0:2, px1])
            nc.scalar.dma_start(out=xg1[:, 2:4, 0:g1w], in_=feat_k[:, 2:4, px1])

            px0 = 0
            for g, gw in enumerate(GROUPS):
                px = slice(px0, px0 + gw)
                if g == 0:
                    xg = xg0
                elif g == 1:
                    xg = xg1
                else:
                    xg = xin.tile([128, KCH, GW], dt.float32r, tag="x")
                    q = nc.sync if g % 2 == 0 else nc.gpsimd
                    q.dma_start(out=xg[:, 0:2, 0:gw], in_=feat_k[:, 0:2, px])
                    q.dma_start(out=xg[:, 2:4, 0:gw], in_=feat_k[:, 2:4, px])

                segs = [
                    slice(t * 512, min((t + 1) * 512, gw))
                    for t in range((gw + 511) // 512)
                ]
                # squared chunks, reduced as a 2-engine pair tree:
                #   DVE:    d0 = sq0+sq1 ; d1 = sq2+sq3 ; d0 += d1
                #   GpSimd: g0 = sq4+sq5 ; g1 = sq6+sq7 ; g0 += g1
                #   DVE:    acc = d0 + g0
                pair_res = [None] * 4
                for m in range(MCH):
                    pst = ps.tile([128, GW], dt.float32, tag="mm")
                    for k in range(KCH):
                        for cs in segs:
                            nc.tensor.matmul(
                                out=pst[:, cs], lhsT=ws[m][:, k, :],
                                rhs=xg[:, k, cs],
                                start=(k == 0), stop=(k == KCH - 1),
                            )
                    half = "ab"[m // 4]
                    sq = sqp.tile([128, GW], dt.float32, tag=f"sq{half}{m % 2}")
                    nc.scalar.activation(
                        out=sq[:, 0:gw], in_=pst[:, 0:gw], func=Act.Square,
                        bias=negc_sb[:, m : m + 1], scale=1.0,
                    )
                    if m % 2 == 1:
                        pr = redp.tile([128, GW], dt.float32, tag=f"pr{half}")
                        eng = nc.vector if m < 4 else nc.gpsimd
                        eng.tensor_add(
                            out=pr[:, 0:gw], in0=pair_res_in[:, 0:gw],
                            in1=sq[:, 0:gw],
                        )
                        pair_res[m // 2] = pr
                    else:
                        pair_res_in = sq
                nc.vector.tensor_add(
                    out=pair_res[0][:, 0:gw], in0=pair_res[0][:, 0:gw],
                    in1=pair_res[1][:, 0:gw],
                )
                nc.gpsimd.tensor_add(
                    out=pair_res[2][:, 0:gw], in0=pair_res[2][:, 0:gw],
                    in1=pair_res[3][:, 0:gw],
                )
                acc = accp.tile([128, GW], dt.float32, tag="acc")
                nc.vector.tensor_add(
                    out=acc[:, 0:gw], in0=pair_res[0][:, 0:gw],
                    in1=pair_res[2][:, 0:gw],
                )

                tmp = outp.tile([64, GW], dt.float32, tag="tmp")
                nc.vector.tensor_copy(out=tmp[:, 0:gw], in_=acc[64:128, 0:gw])
                hc = outp.tile([64, GW], dt.float32, tag="hc")
                nc.vector.tensor_add(
                    out=hc[:, 0:gw], in0=acc[0:64, 0:gw], in1=tmp[:, 0:gw]
                )
                eo = outp.tile([64, GW], dt.float32, tag="eo")
                nc.scalar.activation(
                    out=eo[:, 0:gw], in_=hc[:, 0:gw], func=Act.Exp,
                    bias=0.0, scale=EXP_SCALE,
                )
                nc.scalar.dma_start(out=out_d[:, px], in_=eo[:, 0:gw])
                px0 += gw

    nc.finalize()
    return nc


_NC_CACHE = {}


def _get_nc():
    if "nc" not in _NC_CACHE:
        _NC_CACHE["nc"] = _build()
    return _NC_CACHE["nc"]


def _prep_inputs(features, weights, m, N):
    # wt[f, e*64+c] = weights[e, c, f]
    wt = np.ascontiguousarray(
        weights.astype(np.float32).transpose(2, 0, 1).reshape(F, E * C)
    )
    cent = (m.astype(np.float32) / N.astype(np.float32)[None, :]).reshape(-1)  # [ec]
    negc = np.ascontiguousarray(-cent.reshape(MCH, 128).T)  # [128, MCH]
    feats = np.ascontiguousarray(features.astype(np.float32).reshape(B, F, PIX))
    return [{"feat": feats[i], "wt": wt, "negc": negc} for i in range(B)]


def run_spmd(features, weights, m, N, trace=False):
    in_maps = _prep_inputs(features, weights, m, N)
    res = run_bass_kernel_spmd(_get_nc(), in_maps, list(range(B)), trace=trace)
    out = np.stack([res.results[i]["out"] for i in range(B)])  # [B, C, PIX]
    return out.reshape(B, C, H, W).astype(np.float32), res


def kernel(features, weights, m, N):
    out, _ = run_spmd(features, weights, m, N, trace=False)
    return out


# revision 5
# speedup vs baseline: 1.0397x; 1.0397x over previous
"""DUQ RBF head kernel for Trainium2 (8 NeuronCores, batch-parallel).

Computes out[b,c,h,w] = exp(gamma * mean_e (einsum('bfhw,ecf', x, W) - m/N)^2)
for features [8,512,128,128], weights [16,64,512], m [16,64], N [64].

Strategy: data-parallel over batch (1 image per core). Per core, one big
matmul [ec=1024, f=512] @ [f=512, pix=16384] in float32r (full-rate fp32,
self-loading weights; LDWEIGHTS hides behind the previous matmul's
streaming). Pixels are processed in groups of up to 1024 (two 512-col
PSUM banks per ec-chunk); the Square epilogue folds the centroid into the
per-partition ACT bias, DVE accumulates the 8 squared ec-chunks and folds
the two e-halves, ACT applies the final Exp. N per matmul stays >= 256:
fp32r drops to 1/4 rate below 256 moving columns.

Schedule shaping (vs the flat-group baseline):
- Groups taper up [256, 512, 1024...] so the first matmul only waits for
  a 512KB x-group and one 256KB weight tile, and taper down
  [..., 512, 512, 256] so the serial epilogue chain after the last
  matmul is half-width.
- All 8 weight tiles ride the sync HWDGE ring back-to-back (HWDGE has
  ~0.6us first-byte latency; the gpsimd SWDGE path costs ~2us per DMA
  and starves the early m-chunks). x groups 0-2 ride the scalar HWDGE
  ring; steady-state x groups alternate sync/gpsimd where the 4-group
  pipeline depth hides the SWDGE latency. GpSimd does no elementwise
  work: its adds are 2x slower than DVE and back-pressure the PSUM ring.
"""

import numpy as np

import concourse.bacc as bacc_mod
import concourse.mybir as mybir
import concourse.tile as tile
from concourse.bass_utils import run_bass_kernel_spmd

dt = mybir.dt
Act = mybir.ActivationFunctionType

B, F, H, W = 8, 512, 128, 128
E, C = 16, 64
PIX = H * W           # 16384 pixels per image
MCH = (E * C) // 128  # 8 ec-chunks of 128 partitions
KCH = F // 128        # 4 contraction chunks
LENGTH_SCALE = 0.1
GAMMA = -1.0 / (2.0 * LENGTH_SCALE**2)   # -50.0
EXP_SCALE = GAMMA / E                    # -3.125

GROUPS = [256, 512] + [1024] * 14 + [512, 512, 256]
assert sum(GROUPS) == PIX
assert all(g >= 256 for g in GROUPS)  # fp32r is 1/4-rate below 256 cols
GW = 1024  # allocation width; narrower groups use [:, :gw] slices


def _build():
    nc = bacc_mod.Bacc(None)
    feat_d = nc.declare_dram_parameter("feat", [F, PIX], dt.float32r, isOutput=False)
    wt_d = nc.declare_dram_parameter("wt", [F, E * C], dt.float32r, isOutput=False)
    negc_d = nc.declare_dram_parameter("negc", [128, MCH], dt.float32, isOutput=False)
    out_d = nc.declare_dram_parameter("out", [C, PIX], dt.float32, isOutput=True)

    feat_k = feat_d.rearrange("(k p) x -> p k x", k=KCH)
    wt_k = wt_d.rearrange("(k p) m -> p k m", k=KCH)

    with tile.TileContext(nc) as tc:
        with (
            tc.tile_pool(name="singles", bufs=1) as singles,
            tc.tile_pool(name="xin", bufs=4) as xin,
            tc.tile_pool(name="sqp", bufs=2) as sqp,
            tc.tile_pool(name="accp", bufs=2) as accp,
            tc.tile_pool(name="outp", bufs=2) as outp,
            tc.tile_pool(name="ps", bufs=4, space="PSUM") as ps,
        ):
            # negc rides the (otherwise idle early) gpsimd queue
            negc_sb = singles.tile([128, MCH], dt.float32, tag="negc")
            nc.gpsimd.dma_start(out=negc_sb, in_=negc_d[:, :])

            ws = []
            for m in range(MCH):
                wsm = singles.tile([128, KCH, 128], dt.float32r, tag=f"ws{m}")
                ws.append(wsm)

            # Startup wave: weights back-to-back on sync (HWDGE), x groups
            # 0-2 on scalar (HWDGE).
            g0w, g1w = GROUPS[0], GROUPS[1]
            xg0 = xin.tile([128, KCH, GW], dt.float32r, tag="x")
            xg1 = xin.tile([128, KCH, GW], dt.float32r, tag="x")
            nc.sync.dma_start(out=ws[0], in_=wt_k[:, :, 0:128])
            nc.scalar.dma_start(out=xg0[:, :, 0:g0w], in_=feat_k[:, :, 0:g0w])
            for m in range(1, MCH):
                nc.sync.dma_start(out=ws[m], in_=wt_k[:, :, m * 128 : (m + 1) * 128])
            px1 = slice(g0w, g0w + g1w)
            nc.scalar.dma_start(out=xg1[:, 0:2, 0:g1w], in_=feat_k[:, 0:2, px1])
            nc.scalar.dma_start(out=xg1[:, 2:4, 0:g1w], in_=feat_k[:, 2:4, px1])

            px0 = 0
            for g, gw in enumerate(GROUPS):
                px = slice(px0, px0 + gw)
                if g == 0:
                    xg = xg0
                elif g == 1:
                    xg = xg1
                else:
                    xg = xin.tile([128, KCH, GW], dt.float32r, tag="x")
                    if g == 2:
                        q = nc.scalar
                    else:
                        q = nc.sync if g % 2 == 0 else nc.gpsimd
                    q.dma_start(out=xg[:, 0:2, 0:gw], in_=feat_k[:, 0:2, px])
                    q.dma_start(out=xg[:, 2:4, 0:gw], in_=feat_k[:, 2:4, px])

                segs = [
                    slice(t * 512, min((t + 1) * 512, gw))
                    for t in range((gw + 511) // 512)
                ]
                acc = accp.tile([128, GW], dt.float32, tag="acc")
                for m in range(MCH):
                    pst = ps.tile([128, GW], dt.float32, tag="mm")
                    for k in range(KCH):
                        for cs in segs:
                            nc.tensor.matmul(
                                out=pst[:, cs], lhsT=ws[m][:, k, :],
                                rhs=xg[:, k, cs],
                                start=(k == 0), stop=(k == KCH - 1),
                            )
                    if m == 0:
                        nc.scalar.activation(
                            out=acc[:, 0:gw], in_=pst[:, 0:gw], func=Act.Square,
                            bias=negc_sb[:, 0:1], scale=1.0,
                        )
                    else:
                        sq = sqp.tile([128, GW], dt.float32, tag="sq")
                        nc.scalar.activation(
                            out=sq[:, 0:gw], in_=pst[:, 0:gw], func=Act.Square,
                            bias=negc_sb[:, m : m + 1], scale=1.0,
                        )
                        nc.vector.tensor_add(
                            out=acc[:, 0:gw], in0=acc[:, 0:gw], in1=sq[:, 0:gw]
                        )

                tmp = outp.tile([64, GW], dt.float32, tag="tmp")
                nc.vector.tensor_copy(out=tmp[:, 0:gw], in_=acc[64:128, 0:gw])
                hc = outp.tile([64, GW], dt.float32, tag="hc")
                nc.vector.tensor_add(
                    out=hc[:, 0:gw], in0=acc[0:64, 0:gw], in1=tmp[:, 0:gw]
                )
                eo = outp.tile([64, GW], dt.float32, tag="eo")
                nc.scalar.activation(
                    out=eo[:, 0:gw], in_=hc[:, 0:gw], func=Act.Exp,
                    bias=0.0, scale=EXP_SCALE,
                )
                nc.scalar.dma_start(out=out_d[:, px], in_=eo[:, 0:gw])
                px0 += gw

    nc.finalize()
    return nc


_NC_CACHE = {}


def _get_nc():
    if "nc" not in _NC_CACHE:
        _NC_CACHE["nc"] = _build()
    return _NC_CACHE["nc"]


def _prep_inputs(features, weights, m, N):
    # wt[f, e*64+c] = weights[e, c, f]
    wt = np.ascontiguousarray(
        weights.astype(np.float32).transpose(2, 0, 1).reshape(F, E * C)
    )
    cent = (m.astype(np.float32) / N.astype(np.float32)[None, :]).reshape(-1)  # [ec]
    negc = np.ascontiguousarray(-cent.reshape(MCH, 128).T)  # [128, MCH]
    feats = np.ascontiguousarray(features.astype(np.float32).reshape(B, F, PIX))
    return [{"feat": feats[i], "wt": wt, "negc": negc} for i in range(B)]


def run_spmd(features, weights, m, N, trace=False):
    in_maps = _prep_inputs(features, weights, m, N)
    res = run_bass_kernel_spmd(_get_nc(), in_maps, list(range(B)), trace=trace)
    out = np.stack([res.results[i]["out"] for i in range(B)])  # [B, C, PIX]
    return out.reshape(B, C, H, W).astype(np.float32), res


def kernel(features, weights, m, N):
    out, _ = run_spmd(features, weights, m, N, trace=False)
    return out
